# revision 3
# baseline (speedup 1.0000x reference)
"""DistinctionLoss Trainium2 kernel v2 (raw bacc, hand-scheduled).

Math (per batch b, one batch per core):
  f_n = x_n / ||x_n||                       (row-normalized features)
  s   = sum_n f_n                           ([D] weighted row sum)
  mean(gram) = ||s||^2 / N^2                (the N x N gram is never built)
  dot_n = f_n . s = rn_n * (x_n . s)
  sim_n = (dot_n - 1)/(N-1);  t_n = 1 - relu(sim_n)
  bce  = -mean(t*log(sc) + (1-t)*log1p(-sc))   (logs clamped at -100)
       = -mean(ls - relu(sim)*w),  w = ls - l1
  loss = bce + 1 - mean_b(||s_b||^2)/N^2

v2 schedule vs v1:
  - x DMA'd in 5 big chunks on the sync HWDGE queue (4KB descriptors);
    scores on the gpsimd SWDGE queue.
  - pass1 (ssq per row) via DVE scalar_tensor_tensor (bypass/mult+accum)
    per group, with a few trailing groups per chunk offloaded to ACT
    Square+accum; overlapped with DMA.
  - s accumulated on PE (32 matmuls, rn column stationary); PE pre-warmed
    with dummy matmuls so the real ones run at 2.4 GHz.
  - pass2 (x . s per row) via 32 DVE TTR against the broadcast s.
  - BCE tail on DVE; single final cross-partition matmul; host does the
    tiny final reduction over the 8 per-core outputs.
"""

import os

import numpy as np
import ml_dtypes

B = 8
N, D, P = 4096, 256, 128
G = N // P
CH = [8, 8, 8, 6, 2]
NCH = len(CH)
OFF = [sum(CH[:i]) for i in range(NCH)]
ACT_SQ = ([0] * 5 if os.environ.get("V2_NO_ACTSQ")
          else [2, 2, 2, 2, 0])  # trailing groups per chunk squared on ACT
NINV = 1.0 / (N - 1)
LOG_CLAMP = -100.0
N_WARM = 0 if os.environ.get("V2_NO_WARM") else 24
NO_SSQ = bool(os.environ.get("V2_NO_SSQ"))
PHASE_A = bool(os.environ.get("V2_PHASE_A"))

_cache = {}


def _build_nc():
    import concourse.bacc as bacc
    from concourse import mybir
    from contextlib import ExitStack

    fp32 = mybir.dt.float32
    bf16 = mybir.dt.bfloat16
    AF = mybir.ActivationFunctionType
    ALU = mybir.AluOpType
    AX = mybir.AxisListType

    nc = bacc.Bacc(
        "TRN2", target_bir_lowering=False, debug=False,
        enable_asserts=False, num_devices=8,
    )

    xd = nc.dram_tensor("xbf", [P, G * D], bf16, kind="ExternalInput")
    scd = nc.dram_tensor("scores", [P, G], fp32, kind="ExternalInput")
    out_d = nc.dram_tensor("out", [1, 2], fp32, kind="ExternalOutput")

    sb = nc.alloc_sbuf_tensor
    x_t = sb("x", [P, G, D], bf16)
    pt_t = sb("pt", [P, G, D], bf16)     # DVE TTR product sink (per group)
    sqa_t = sb("sqa", [P, 8, D], bf16)   # ACT square sinks (per square)
    ssq_t = sb("ssq", [P, G], fp32)
    issq_t = sb("issq", [P, G], fp32)
    rnbf_t = sb("rnbf", [P, G], bf16)
    sc_t = sb("sc", [P, G], fp32)
    ls_t = sb("ls", [P, G], fp32)
    l1_t = sb("l1", [P, G], fp32)
    w_t = sb("w", [P, G], fp32)
    lssum_t = sb("lssum", [P, 1], fp32)
    draw_t = sb("draw", [P, G], fp32)
    dots_t = sb("dots", [P, G], fp32)
    sim_t = sb("sim", [P, G], fp32)
    rterm_t = sb("rterm", [P, G], fp32)
    rwsum_t = sb("rwsum", [P, 1], fp32)
    onesb_t = sb("onesb", [1, P], bf16)
    onesf_t = sb("onesf", [P, 1], fp32)
    sbf1_t = sb("sbf1", [1, D], bf16)
    sbc_t = sb("sbc", [P, D], bf16)
    outsb_t = sb("outsb", [P, 2], fp32)
    outfin_t = sb("outfin", [1, 2], fp32)
    warm_t = sb("warm", [1, 3], fp32)

    ctx = ExitStack()
    ps_s = ctx.enter_context(nc.psum_tensor([1, D], fp32))
    ps_bc = ctx.enter_context(nc.psum_tensor([P, D], fp32))
    ps_tot = ctx.enter_context(nc.psum_tensor([1, 2], fp32))
    names = ([f"S_dx{k}" for k in range(NCH)] +
             ["S_dsc", "S_ln", "S_sqa", "S_issq", "S_rnbf", "S_pe",
              "S_sbf", "S_pebc", "S_sbc", "S_dve", "S_pef", "S_fin",
              "S_ones", "S_od"])
    S = {n: ctx.enter_context(nc.semaphore(n)) for n in names}
    S_dx = [S[f"S_dx{k}"] for k in range(NCH)]

    def gsl(k):
        return slice(OFF[k], OFF[k] + CH[k])

    with ctx, nc.Block() as block:
        @block.sync
        def _(sync):
            for k in range(NCH):
                sync.dma_start(
                    out=x_t[:, gsl(k), :],
                    in_=xd[:, OFF[k] * D:(OFF[k] + CH[k]) * D],
                ).then_inc(S_dx[k], 16)
            sync.wait_ge(S["S_fin"], 1)
            sync.dma_start(out=out_d[:], in_=outfin_t[:]).then_inc(S["S_od"], 16)
            sync.wait_ge(S["S_od"], 16)

        @block.gpsimd
        def _(gp):
            gp.dma_start(out=sc_t[:], in_=scd[:]).then_inc(S["S_dsc"], 16)

        @block.scalar
        def _(act):
            # front-load the three ACT tables in need-order
            act.activation(out=warm_t[:, 0:1],
                           in_=nc.const_aps.tensor(1.0, (1, 1)), func=AF.Square)
            act.sqrt(warm_t[:, 1:2], nc.const_aps.tensor(1.0, (1, 1)))
            act.activation(out=warm_t[:, 2:3],
                           in_=nc.const_aps.tensor(1.0, (1, 1)), func=AF.Ln)
            for k in range(NCH):
                na = ACT_SQ[k]
                if na:
                    act.wait_ge(S_dx[k], 16)
                    mm = None
                    for j in range(na):
                        g = OFF[k] + CH[k] - na + j
                        mm = act.activation(
                            out=sqa_t[:, 2 * k + j, :], in_=x_t[:, g, :],
                            func=AF.Square,
                            accum_out=ssq_t[:, g:g + 1],
                        )
                    mm.then_inc(S["S_sqa"], 1)
                act.wait_ge(S["S_issq"], k + 1)
                act.sqrt(rnbf_t[:, gsl(k)], issq_t[:, gsl(k)]
                         ).then_inc(S["S_rnbf"], 1)
                if k == 0:
                    # scores logs ride behind chunk 0's work
                    act.wait_ge(S["S_dsc"], 16)
                    act.activation(out=ls_t[:], in_=sc_t[:], func=AF.Ln)
                    act.activation(out=l1_t[:], in_=sc_t[:], func=AF.Ln,
                                   scale=-1.0, bias=1.0).then_inc(S["S_ln"], 1)
            if PHASE_A:
                act.copy(outfin_t[:], ssq_t[0:1, 0:2]).then_inc(S["S_fin"], 1)
            else:
                act.wait_ge(S["S_pe"], 1)
                act.copy(sbf1_t[:], ps_s[:]).then_inc(S["S_sbf"], 1)
                act.wait_ge(S["S_pebc"], 1)
                act.copy(sbc_t[:], ps_bc[:]).then_inc(S["S_sbc"], 1)
                act.wait_ge(S["S_pef"], 1)
                act.copy(outfin_t[:], ps_tot[:]).then_inc(S["S_fin"], 1)

        @block.vector
        def _(dve):
            dve.memset(onesb_t[:], 1.0)
            dve.memset(onesf_t[:], 1.0)
            dve.memset(outsb_t[:], 0.0).then_inc(S["S_ones"], 1)
            nsq = 0
            for k in range(NCH):
                dve.wait_ge(S_dx[k], 16)
                for g in range(OFF[k], OFF[k] + CH[k] - ACT_SQ[k]):
                    dve.scalar_tensor_tensor(
                        out=pt_t[:, g, :], in0=x_t[:, g, :], scalar=0.0,
                        in1=x_t[:, g, :], op0=ALU.bypass, op1=ALU.mult,
                        accum_out=ssq_t[:, g:g + 1],
                    )
                if ACT_SQ[k]:
                    nsq += 1
                    dve.wait_ge(S["S_sqa"], nsq)
                dve.drain()
                dve.reciprocal(issq_t[:, gsl(k)], ssq_t[:, gsl(k)]
                               ).then_inc(S["S_issq"], 1)
                if k == 1:
                    # scores tail rides behind chunk 1 (S_ln long since up)
                    dve.wait_ge(S["S_ln"], 1)
                    dve.tensor_scalar_max(ls_t[:], ls_t[:], LOG_CLAMP)
                    dve.drain()
                    dve.tensor_sub(w_t[:], ls_t[:], l1_t[:])
                    dve.tensor_reduce(out=lssum_t[:], in_=ls_t[:], axis=AX.X,
                                      op=ALU.add)
            # ||s||^2 from the bf16 SBUF copy of s (error ~1e-9 on the loss)
            if PHASE_A:
                return
            dve.wait_ge(S["S_sbf"], 1)
            dve.scalar_tensor_tensor(
                out=pt_t[0:1, 0, :], in0=sbf1_t[:], scalar=0.0,
                in1=sbf1_t[:], op0=ALU.bypass, op1=ALU.mult,
                accum_out=outsb_t[0:1, 1:2],
            )
            dve.drain()
            # pass2: per-row dot with broadcast s
            dve.wait_ge(S["S_sbc"], 1)
            for g in range(G):
                dve.scalar_tensor_tensor(
                    out=pt_t[:, g, :], in0=x_t[:, g, :], scalar=0.0,
                    in1=sbc_t[:], op0=ALU.bypass, op1=ALU.mult,
                    accum_out=draw_t[:, g:g + 1],
                )
            dve.drain()
            dve.tensor_mul(dots_t[:], draw_t[:], rnbf_t[:])
            dve.drain()
            dve.tensor_scalar(
                out=sim_t[:], in0=dots_t[:], scalar1=1.0, scalar2=NINV,
                op0=ALU.subtract, op1=ALU.mult,
            )
            dve.drain()
            dve.scalar_tensor_tensor(
                out=rterm_t[:], in0=sim_t[:], scalar=0.0, in1=w_t[:],
                op0=ALU.max, op1=ALU.mult, accum_out=rwsum_t[:],
            )
            dve.drain()
            dve.tensor_sub(outsb_t[:, 0:1], lssum_t[:], rwsum_t[:]
                           ).then_inc(S["S_dve"], 1)

        @block.tensor
        def _(pe):
            if PHASE_A:
                pe.wait_ge(S["S_ones"], 1)
                return
            # keep PE busy from the start so HAM unthrottles before the
            # real accumulation matmuls
            pe.wait_ge(S["S_ones"], 1)
            for _ in range(N_WARM):
                pe.matmul(ps_bc[:, 0:P], onesb_t[:], onesb_t[:],
                          start=True, stop=True)
            mm = None
            for k in range(NCH):
                pe.wait_ge(S["S_rnbf"], k + 1)
                for g in range(OFF[k], OFF[k] + CH[k]):
                    mm = pe.matmul(
                        ps_s[:], rnbf_t[:, g:g + 1], x_t[:, g, :],
                        start=(g == 0), stop=(g == G - 1),
                    )
            mm.then_inc(S["S_pe"], 1)
            pe.wait_ge(S["S_sbf"], 1)
            pe.matmul(ps_bc[:], onesb_t[:], sbf1_t[:], start=True, stop=True
                      ).then_inc(S["S_pebc"], 1)
            pe.wait_ge(S["S_dve"], 1)
            pe.matmul(ps_tot[:], onesf_t[:], outsb_t[:], start=True, stop=True
                      ).then_inc(S["S_pef"], 1)

    nc.finalize()
    return nc


def _get_nc():
    if "nc" not in _cache:
        _cache["nc"] = _build_nc()
    return _cache["nc"]


def run_on_device(features: np.ndarray, scores: np.ndarray, trace: bool = False,
                  tmpdir: str | None = None):
    """Returns (per_core_outputs [8, 2] float64, BassKernelResults)."""
    from concourse.bass_utils import run_bass_kernel_spmd

    nc = _get_nc()
    in_maps = []
    for c in range(B):
        in_maps.append({
            "xbf": np.ascontiguousarray(features[c]).reshape(P, G * D)
            .astype(ml_dtypes.bfloat16),
            "scores": np.ascontiguousarray(scores[c]).reshape(P, G)
            .astype(np.float32),
        })
    res = run_bass_kernel_spmd(nc, in_maps, core_ids=list(range(B)),
                               trace=trace, tmpdir=tmpdir)
    outs = np.stack([res.results[c]["out"].reshape(2) for c in range(B)])
    return outs.astype(np.float64), res


def kernel(features: np.ndarray, scores: np.ndarray) -> np.ndarray:
    outs, _ = run_on_device(features, scores)
    bce_sums = outs[:, 0]                         # per-batch sum(ls - relu*w)
    ssqs = outs[:, 1]                             # per-batch ||s||^2
    bce = np.mean(-bce_sums / N)
    feat = 1.0 - np.sum(ssqs) / (B * float(N) * float(N))
    return np.asarray(bce + feat, dtype=np.float32)
